# revision 19
# baseline (speedup 1.0000x reference)
"""Trainium2 Bass kernel for AntisymmetricRNN scan.

Reference computation (per batch column b, independent chains):
    A   = triu(W,1) - triu(W,1)^T - 0.001*I          (256x256)
    X_0 = X0^T (n=256, bs=256)
    Y_t = A @ X_t + by
    X_{t+1} = X_t + 0.01*tanh(Y_t),  t = 0..998
    out = stack([X_0 .. X_999]) -> (bs, tmax, n) = (256, 1000, 256)

Strategy (data-parallel over batch, 8 cores, bs=32 per core):
  - Keep Y in PSUM as a running accumulator (linearity of A@):
        Y_{t+1} = Y_t + M @ G_t,   M = 0.01*A,  G_t = tanh(Y_t)
  - fp16 everywhere on the matmul path: Mq = fp16(M) as the stationary
    operand, G stored fp16.  The fp16 quantization error of M is fixed
    with a *deferred linear correction*: every KWIN steps apply
        Y += Rs @ (sum of window G)/64,   Rs = fp16(64*(M - Mq))
    which is exact for the dropped linear term (measured on HW: rel
    err 4.2e-3 over 999 steps vs the 2e-2 gate).
  - The device only emits G_t (fp16).  The host reconstructs
        X_t = X_0 + 0.01 * cumsum(G)  (fp32)
    eliminating the per-step X update on DVE and halving DMA traffic.
  - NG=4 independent sub-chain groups per core (bw=8 batch columns
    each, own PSUM bank) software-pipeline the tanh: the in-order ACT
    queue stays ~96% busy because 4 tanh instructions cover one
    group's MM->PSUM->tanh round-trip (~0.7us).
  - HW-measured TANH cost is ~98ns/instruction + ~5ns/element
    (independent of operand dtype/bytes), so the kernel is tanh-bound
    at ~700ns/step: 999 * 4 * 175ns = 698us of ACT engine time.
    NG=2 would halve the fixed cost but cannot cover the round-trip
    (measured lockstep collapse at 728us); NG=4 is the optimum.
  - Per step per group: 1 ACT tanh (PSUM->SBUF slab), 4 fp16 matmul
    accumulates (zigzag chunk order shares boundary LDWEIGHTS; PE is
    LDWEIGHTS-bound at ~42ns/matmul), 1 window G-sum add (split
    DVE/GPSIMD).  G slabs are DMAed to DRAM in RB-step batches.
  - Measured on TRN2: 720.7us vs 2357us for the staged fp32 baseline
    (3.27x).
"""

import numpy as np

N = 256
BS = 256
TMAX = 1000
STEP = 0.01
EPS = 0.001
NCORES = 8
BSH = BS // NCORES  # 32 batch columns per core
NSTEPS = TMAX - 1   # 999 device steps
H = 2               # n-halves (256 = 2 x 128 partitions)

# Tunables
NG = 4              # independent chain groups per core (divides BSH)
KWIN = 64           # steps per deferred-correction window
TRUNC_Y = False     # tanh reads Y as bf16 (high half of each PSUM f32);
                    # measured no speedup (ACT cost is per-element), so
                    # keep the full-precision read and its error margin
RB = 37             # steps per output slab DMA (999 = 27 * 37)
SLAB_BUFS = 3       # output slab buffering depth
GW_BUFS = 2         # gwin (window-sum) tile buffering


def _chunk_order(g):
    order = [(k, m) for k in range(H) for m in range(H)]
    return order if g % 2 == 0 else order[::-1]


def _build_graph(repeat=1):
    import concourse.bass as bass  # noqa
    import concourse.tile as tile
    from concourse import bacc, mybir

    f32 = mybir.dt.float32
    f16 = mybir.dt.float16
    bf16 = mybir.dt.bfloat16
    nc = bacc.Bacc("TRN2", target_bir_lowering=False, debug=False,
                   num_devices=NCORES)

    bw = BSH // NG  # batch columns per group
    gw = H * bw     # group row width (h-major, contiguous)

    mq_d = nc.dram_tensor("mq", [128, 4 * 128], f16, kind="ExternalInput")
    rs_d = nc.dram_tensor("rs", [128, 4 * 128], f16, kind="ExternalInput")
    x0h_d = nc.dram_tensor("x0h", [128, H, BSH], f16, kind="ExternalInput")
    x0l_d = nc.dram_tensor("x0l", [128, H, BSH], f16, kind="ExternalInput")
    x0h64_d = nc.dram_tensor("x0h64", [128, H, BSH], f16,
                             kind="ExternalInput")
    byf_d = nc.dram_tensor("byf", [128, NG, gw], f32, kind="ExternalInput")
    gout_d = nc.dram_tensor("gout", [128, NSTEPS, NG, gw], f16,
                            kind="ExternalOutput")

    with tile.TileContext(nc) as tc:
        with tc.tile_pool(name="const", bufs=1) as cpool, \
             tc.tile_pool(name="gw", bufs=GW_BUFS) as gwpool, \
             tc.tile_pool(name="slab", bufs=SLAB_BUFS) as spool, \
             tc.tile_pool(name="ypsum", bufs=1, space="PSUM") as ypool:

            mq_sb = cpool.tile([128, 4 * 128], f16)
            rs_sb = cpool.tile([128, 4 * 128], f16)
            x0h_sb = cpool.tile([128, H, BSH], f16)
            x0l_sb = cpool.tile([128, H, BSH], f16)
            x0h64_sb = cpool.tile([128, H, BSH], f16)
            byf_sb = cpool.tile([128, NG, gw], f32)

            # Warm the tanh ACT table (1.28us implicit load) during the
            # input-DMA window instead of on the first real step.
            warm = cpool.tile([128, 1], f32)
            nc.vector.memset(warm[:, :], 0.0)
            nc.scalar.activation(warm[:, :], warm[:, :],
                                 mybir.ActivationFunctionType.Tanh)

            nc.sync.dma_start(out=mq_sb[:, :], in_=mq_d[:, :])
            nc.sync.dma_start(out=rs_sb[:, :], in_=rs_d[:, :])
            nc.sync.dma_start(out=x0h_sb[:, :, :], in_=x0h_d[:, :, :])
            nc.sync.dma_start(out=x0l_sb[:, :, :], in_=x0l_d[:, :, :])
            nc.sync.dma_start(out=x0h64_sb[:, :, :], in_=x0h64_d[:, :, :])
            nc.sync.dma_start(out=byf_sb[:, :, :], in_=byf_d[:, :, :])

            # stationary chunk (k, m) of Mq^T / Rs^T (lhsT layout)
            def mch(k, m):
                c = 2 * k + m
                return mq_sb[:, 128 * c:128 * (c + 1)]

            def rch(k, m):
                c = 2 * k + m
                return rs_sb[:, 128 * c:128 * (c + 1)]

            # One full PSUM bank per group ([128, 512] f32 = 2KB/part) so
            # ACT reads of group P never collide with PE writes of Q.
            # Y data sits in the first H*bw columns, h-major: a single
            # contiguous run per partition for the tanh read.
            _yt = [ypool.tile([128, 512], f32, name=f"y{g}")
                   for g in range(NG)]
            ys = [yt[:, 0:gw] for yt in _yt]

            # f16-window G accumulators (persistent).  f16 keeps the DVE
            # 2x mode; the sum of <=16 unit-scale values carries ~1e-3
            # relative noise, harmless on the small correction term.
            gsums = [cpool.tile([128, gw], f16, name=f"gsum{g}")
                     for g in range(NG)]

            # Y_0 = Mq@(X0/h)_hi + Mq@(X0/h)_lo + Rs@((X0/h)_hi/64) + by
            for g in range(NG):
                gsl = slice(g * bw, (g + 1) * bw)
                first = True
                for ch, xs in ((mch, x0h_sb), (mch, x0l_sb),
                               (rch, x0h64_sb)):
                    for k in range(H):
                        for m in range(H):
                            nc.tensor.matmul(
                                ys[g][:, m * bw:(m + 1) * bw],
                                ch(k, m), xs[:, k, gsl],
                                start=first, stop=False,
                                skip_group_check=True)
                            first = False
                nc.vector.tensor_add(ys[g][:, :], ys[g][:, :],
                                     byf_sb[:, g, :])

            for _rep in range(repeat):
                t = 0
                while t < NSTEPS:
                    nb = min(RB, NSTEPS - t)
                    slab = spool.tile([128, RB, NG, gw], f16)
                    for s in range(nb):
                        tt = t + s
                        last = (tt == NSTEPS - 1)
                        for g in range(NG):
                            gtile = slab[:, s, g, :]
                            if TRUNC_Y:
                                # read the high 2 bytes of each PSUM f32
                                # = bf16-truncated Y; halves ACT read
                                # bytes (err 8.8e-3 vs 2.3e-3, gate 2e-2)
                                yin = ys[g][:, :].bitcast(bf16)[:, 1::2]
                            else:
                                yin = ys[g][:, :]
                            nc.scalar.activation(
                                gtile, yin,
                                mybir.ActivationFunctionType.Tanh)
                            if last:
                                continue
                            # Accumulating fp16 matmuls.  Odd groups walk
                            # the weight chunks in reverse (zigzag) so
                            # the chunk at each group boundary is shared
                            # and the PE skips one LDWEIGHTS.
                            for k, m in _chunk_order(g):
                                nc.tensor.matmul(
                                    ys[g][:, m * bw:(m + 1) * bw],
                                    mch(k, m),
                                    slab[:, s, g, k * bw:(k + 1) * bw],
                                    start=False, stop=False,
                                    skip_group_check=True)
                            # Deferred correction: at the last step of a
                            # window, convert the PREVIOUS steps' G-sum
                            # (current step's G not yet added, so this
                            # chain never waits on this step's ACT) and
                            # accumulate Rs@gwin into Y.  Emitted after
                            # the regular matmuls so the in-order PE
                            # never stalls on the DVE-produced gwin.
                            # Window w covers steps [wK-1, wK+K-2].
                            boundary = (tt % KWIN == KWIN - 1)
                            if boundary:
                                gwin = gwpool.tile([128, gw], f16,
                                                   tag=f"gw{g}")
                                nc.vector.tensor_scalar_mul(
                                    gwin[:, :], gsums[g][:, :],
                                    1.0 / 64.0)
                                for k, m in _chunk_order(g):
                                    nc.tensor.matmul(
                                        ys[g][:, m * bw:(m + 1) * bw],
                                        rch(k, m),
                                        gwin[:, k * bw:(k + 1) * bw],
                                        start=False, stop=False,
                                        skip_group_check=True)
                            # window G-sum; a copy resets the window
                            # after each boundary / at t=0.  Split across
                            # DVE (even groups) and the otherwise-idle
                            # GPSIMD (odd groups).
                            veng = nc.vector if g % 2 == 0 else nc.gpsimd
                            if boundary or tt == 0:
                                veng.tensor_copy(gsums[g][:, :], gtile)
                            else:
                                veng.tensor_add(gsums[g][:, :],
                                                gsums[g][:, :], gtile)
                    nc.sync.dma_start(out=gout_d[:, t:t + nb, :, :],
                                      in_=slab[:, :nb, :, :])
                    t += nb

    nc.compile()
    return nc


def _prep_inputs(X0, W, by):
    """Host-side input prep; returns per-core in_maps."""
    X0 = np.asarray(X0, dtype=np.float32)
    W = np.asarray(W, dtype=np.float32)
    by = np.asarray(by, dtype=np.float32).reshape(N, 1)

    U = np.triu(W, 1)
    A = (U - U.T) - np.float32(EPS) * np.eye(N, dtype=np.float32)
    M = (np.float32(STEP) * A).astype(np.float32)
    Mq = M.astype(np.float16)
    Rs = (np.float32(64.0) * (M - Mq.astype(np.float32))).astype(np.float16)

    def pack(mat):
        """lhsT chunks: chunk (k,m) = mat.T[128k:.., 128m:..]."""
        matT = mat.T
        p = np.empty((128, 4 * 128), dtype=np.float16)
        for k in range(H):
            for m in range(H):
                c = 2 * k + m
                p[:, 128 * c:128 * (c + 1)] = \
                    matT[128 * k:128 * (k + 1), 128 * m:128 * (m + 1)]
        return p

    mq_p = pack(Mq)
    rs_p = pack(Rs)

    bw = BSH // NG
    gw = H * bw
    # byf[p, g, m*bw + b] = by[m*128 + p]
    byf = np.empty((128, NG, gw), dtype=np.float32)
    for m in range(H):
        byf[:, :, m * bw:(m + 1) * bw] = by[m * 128:(m + 1) * 128, 0:1][
            :, None, :]

    def fold(arr):
        """[256, BSH] -> [128, H, BSH]."""
        out = np.empty((128, H, BSH), dtype=arr.dtype)
        for h in range(H):
            out[:, h, :] = arr[128 * h:128 * (h + 1), :]
        return out

    in_maps = []
    for c in range(NCORES):
        Xs = (X0[c * BSH:(c + 1) * BSH, :].T
              / np.float32(STEP)).astype(np.float32)  # [n, bsh]
        X0h = Xs.astype(np.float16)
        X0l = (Xs - X0h.astype(np.float32)).astype(np.float16)
        X0h64 = (X0h.astype(np.float32) / np.float32(64.0)).astype(
            np.float16)
        in_maps.append({
            "mq": mq_p,
            "rs": rs_p,
            "x0h": fold(X0h),
            "x0l": fold(X0l),
            "x0h64": fold(X0h64),
            "byf": byf,
        })
    return in_maps


_CACHED_NC = None


def _get_nc():
    global _CACHED_NC
    if _CACHED_NC is None:
        _CACHED_NC = _build_graph()
    return _CACHED_NC


def kernel(X0, W, by, _trace=False, _return_results=False):
    from concourse.bass_utils import run_bass_kernel_spmd

    nc = _get_nc()
    in_maps = _prep_inputs(X0, W, by)
    res = run_bass_kernel_spmd(nc, in_maps, core_ids=list(range(NCORES)),
                               trace=_trace)

    bw = BSH // NG
    X0 = np.asarray(X0, dtype=np.float32)
    out = np.empty((BS, TMAX, N), dtype=np.float32)
    out[:, 0, :] = X0
    for c in range(NCORES):
        arr = res.results[c]["gout"]  # [128, 999, NG, H*bw] fp16
        # p=partition, t, g=group, (m, b) h-major -> G as (b_total, t, n)
        # column index = g*bw + b ; n = m*128 + p
        arr = arr.reshape(128, NSTEPS, NG, H, bw)
        G = np.transpose(arr, (2, 4, 1, 3, 0)).reshape(BSH, NSTEPS, N)
        G = np.cumsum(G.astype(np.float32), axis=1, dtype=np.float32)
        out[c * BSH:(c + 1) * BSH, 1:, :] = (
            X0[c * BSH:(c + 1) * BSH, None, :] + np.float32(STEP) * G)
    if _return_results:
        return out, res
    return out


# revision 21
# speedup vs baseline: 1.1822x; 1.1822x over previous
"""Trainium2 Bass kernel for AntisymmetricRNN scan.

Reference computation (per batch column b, independent chains):
    A   = triu(W,1) - triu(W,1)^T - 0.001*I          (256x256)
    X_0 = X0^T (n=256, bs=256)
    Y_t = A @ X_t + by
    X_{t+1} = X_t + 0.01*tanh(Y_t),  t = 0..998
    out = stack([X_0 .. X_999]) -> (bs, tmax, n) = (256, 1000, 256)

Strategy (data-parallel over batch, 8 cores, bs=32 per core):
  - Keep Y in PSUM as a running accumulator (linearity of A@):
        Y_{t+1} = Y_t + M @ G_t,   M = 0.01*A,  G_t = tanh(Y_t)
  - fp16 everywhere on the matmul path: Mq = fp16(M) as the stationary
    operand, G stored fp16.  The fp16 quantization error of M is fixed
    with a *deferred linear correction*: every KWIN steps apply
        Y += Rs @ (sum of window G)/64,   Rs = fp16(64*(M - Mq))
    which is exact for the dropped linear term (measured on HW: rel
    err 4.2e-3 over 999 steps vs the 2e-2 gate).
  - The device only emits G_t (fp16).  The host reconstructs
        X_t = X_0 + 0.01 * cumsum(G)  (fp32)
    eliminating the per-step X update on DVE and halving DMA traffic.
  - NG=4 independent sub-chain groups per core (bw=8 batch columns
    each, own PSUM bank) software-pipeline the tanh: the in-order ACT
    queue stays ~96% busy because 4 tanh instructions cover one
    group's MM->PSUM->tanh round-trip (~0.7us).
  - HW-measured TANH cost is ~98ns/instruction + ~5ns/element
    (independent of operand dtype/bytes), so the kernel is tanh-bound
    at ~700ns/step: 999 * 4 * 175ns = 698us of ACT engine time.
    NG=2 would halve the fixed cost but cannot cover the round-trip
    (measured lockstep collapse at 728us); NG=4 is the optimum.
  - Per step per group: 1 ACT tanh (PSUM->SBUF slab), 4 fp16 matmul
    accumulates (zigzag chunk order shares boundary LDWEIGHTS; PE is
    LDWEIGHTS-bound at ~42ns/matmul), 1 window G-sum add (split
    DVE/GPSIMD).  G slabs are DMAed to DRAM in RB-step batches.
  - Measured on TRN2: 720.7us vs 2357us for the staged fp32 baseline
    (3.27x).
"""

import numpy as np

N = 256
BS = 256
TMAX = 1000
STEP = 0.01
EPS = 0.001
NCORES = 8
BSH = BS // NCORES  # 32 batch columns per core
NSTEPS = TMAX - 1   # 999 device steps
H = 2               # n-halves (256 = 2 x 128 partitions)

# Tunables
NG = 4              # independent chain groups per core (divides BSH)
KWIN = 32           # steps per deferred-correction window
TRUNC_Y = False     # tanh reads Y as bf16 (high half of each PSUM f32);
                    # measured no speedup (ACT cost is per-element), so
                    # keep the full-precision read and its error margin
RB = 37             # steps per output slab DMA (999 = 27 * 37)
SLAB_BUFS = 3       # output slab buffering depth
GW_BUFS = 2         # gwin (window-sum) tile buffering


def _chunk_order(g):
    order = [(k, m) for k in range(H) for m in range(H)]
    return order if g % 2 == 0 else order[::-1]


def _build_graph(repeat=1):
    import concourse.bass as bass  # noqa
    import concourse.tile as tile
    from concourse import bacc, mybir

    f32 = mybir.dt.float32
    f16 = mybir.dt.float16
    bf16 = mybir.dt.bfloat16
    nc = bacc.Bacc("TRN2", target_bir_lowering=False, debug=False,
                   num_devices=NCORES)

    bw = BSH // NG  # batch columns per group
    gw = H * bw     # group row width (h-major, contiguous)

    mq_d = nc.dram_tensor("mq", [128, 4 * 128], f16, kind="ExternalInput")
    rs_d = nc.dram_tensor("rs", [128, 4 * 128], f16, kind="ExternalInput")
    x0h_d = nc.dram_tensor("x0h", [128, H, BSH], f16, kind="ExternalInput")
    x0l_d = nc.dram_tensor("x0l", [128, H, BSH], f16, kind="ExternalInput")
    x0h64_d = nc.dram_tensor("x0h64", [128, H, BSH], f16,
                             kind="ExternalInput")
    byf_d = nc.dram_tensor("byf", [128, NG, gw], f32, kind="ExternalInput")
    gout_d = nc.dram_tensor("gout", [128, NSTEPS, NG, gw], f16,
                            kind="ExternalOutput")

    with tile.TileContext(nc) as tc:
        with tc.tile_pool(name="const", bufs=1) as cpool, \
             tc.tile_pool(name="gw", bufs=GW_BUFS) as gwpool, \
             tc.tile_pool(name="slab", bufs=SLAB_BUFS) as spool, \
             tc.tile_pool(name="ypsum", bufs=1, space="PSUM") as ypool:

            mq_sb = cpool.tile([128, 4 * 128], f16)
            rs_sb = cpool.tile([128, 4 * 128], f16)
            x0h_sb = cpool.tile([128, H, BSH], f16)
            x0l_sb = cpool.tile([128, H, BSH], f16)
            x0h64_sb = cpool.tile([128, H, BSH], f16)
            byf_sb = cpool.tile([128, NG, gw], f32)

            nc.sync.dma_start(out=mq_sb[:, :], in_=mq_d[:, :])
            nc.sync.dma_start(out=rs_sb[:, :], in_=rs_d[:, :])
            nc.sync.dma_start(out=x0h_sb[:, :, :], in_=x0h_d[:, :, :])
            nc.sync.dma_start(out=x0l_sb[:, :, :], in_=x0l_d[:, :, :])
            nc.sync.dma_start(out=x0h64_sb[:, :, :], in_=x0h64_d[:, :, :])
            nc.sync.dma_start(out=byf_sb[:, :, :], in_=byf_d[:, :, :])

            # stationary chunk (k, m) of Mq^T / Rs^T (lhsT layout)
            def mch(k, m):
                c = 2 * k + m
                return mq_sb[:, 128 * c:128 * (c + 1)]

            def rch(k, m):
                c = 2 * k + m
                return rs_sb[:, 128 * c:128 * (c + 1)]

            # One full PSUM bank per group ([128, 512] f32 = 2KB/part) so
            # ACT reads of group P never collide with PE writes of Q.
            # Y data sits in the first H*bw columns, h-major: a single
            # contiguous run per partition for the tanh read.
            _yt = [ypool.tile([128, 512], f32, name=f"y{g}")
                   for g in range(NG)]
            ys = [yt[:, 0:gw] for yt in _yt]

            # f16-window G accumulators (persistent).  f16 keeps the DVE
            # 2x mode; the sum of <=16 unit-scale values carries ~1e-3
            # relative noise, harmless on the small correction term.
            gsums = [cpool.tile([128, gw], f16, name=f"gsum{g}")
                     for g in range(NG)]

            # Y_0 = Mq@(X0/h)_hi + Mq@(X0/h)_lo + Rs@((X0/h)_hi/64) + by
            for g in range(NG):
                gsl = slice(g * bw, (g + 1) * bw)
                first = True
                for ch, xs in ((mch, x0h_sb), (mch, x0l_sb),
                               (rch, x0h64_sb)):
                    for k in range(H):
                        for m in range(H):
                            nc.tensor.matmul(
                                ys[g][:, m * bw:(m + 1) * bw],
                                ch(k, m), xs[:, k, gsl],
                                start=first, stop=False,
                                skip_group_check=True)
                            first = False
                nc.vector.tensor_add(ys[g][:, :], ys[g][:, :],
                                     byf_sb[:, g, :])

            for _rep in range(repeat):
                t = 0
                while t < NSTEPS:
                    nb = min(RB, NSTEPS - t)
                    slab = spool.tile([128, RB, NG, gw], f16)
                    for s in range(nb):
                        tt = t + s
                        last = (tt == NSTEPS - 1)
                        for g in range(NG):
                            gtile = slab[:, s, g, :]
                            if TRUNC_Y:
                                # read the high 2 bytes of each PSUM f32
                                # = bf16-truncated Y; halves ACT read
                                # bytes (err 8.8e-3 vs 2.3e-3, gate 2e-2)
                                yin = ys[g][:, :].bitcast(bf16)[:, 1::2]
                            else:
                                yin = ys[g][:, :]
                            nc.scalar.activation(
                                gtile, yin,
                                mybir.ActivationFunctionType.Tanh)
                            if last:
                                continue
                            # Accumulating fp16 matmuls.  Odd groups walk
                            # the weight chunks in reverse (zigzag) so
                            # the chunk at each group boundary is shared
                            # and the PE skips one LDWEIGHTS.
                            for k, m in _chunk_order(g):
                                nc.tensor.matmul(
                                    ys[g][:, m * bw:(m + 1) * bw],
                                    mch(k, m),
                                    slab[:, s, g, k * bw:(k + 1) * bw],
                                    start=False, stop=False,
                                    skip_group_check=True)
                            # Deferred correction: at the last step of a
                            # window, convert the PREVIOUS steps' G-sum
                            # (current step's G not yet added, so this
                            # chain never waits on this step's ACT) and
                            # accumulate Rs@gwin into Y.  Emitted after
                            # the regular matmuls so the in-order PE
                            # never stalls on the DVE-produced gwin.
                            # Window w covers steps [wK-1, wK+K-2].
                            boundary = (tt % KWIN == KWIN - 1)
                            if boundary:
                                gwin = gwpool.tile([128, gw], f16,
                                                   tag=f"gw{g}")
                                nc.vector.tensor_scalar_mul(
                                    gwin[:, :], gsums[g][:, :],
                                    1.0 / 64.0)
                                for k, m in _chunk_order(g):
                                    nc.tensor.matmul(
                                        ys[g][:, m * bw:(m + 1) * bw],
                                        rch(k, m),
                                        gwin[:, k * bw:(k + 1) * bw],
                                        start=False, stop=False,
                                        skip_group_check=True)
                            # window G-sum; a copy resets the window
                            # after each boundary / at t=0.  Split across
                            # DVE (even groups) and the otherwise-idle
                            # GPSIMD (odd groups).
                            veng = nc.vector if g % 2 == 0 else nc.gpsimd
                            if boundary or tt == 0:
                                veng.tensor_copy(gsums[g][:, :], gtile)
                            else:
                                veng.tensor_add(gsums[g][:, :],
                                                gsums[g][:, :], gtile)
                    nc.sync.dma_start(out=gout_d[:, t:t + nb, :, :],
                                      in_=slab[:, :nb, :, :])
                    t += nb

    nc.compile()
    return nc


def _prep_inputs(X0, W, by):
    """Host-side input prep; returns per-core in_maps."""
    X0 = np.asarray(X0, dtype=np.float32)
    W = np.asarray(W, dtype=np.float32)
    by = np.asarray(by, dtype=np.float32).reshape(N, 1)

    U = np.triu(W, 1)
    A = (U - U.T) - np.float32(EPS) * np.eye(N, dtype=np.float32)
    M = (np.float32(STEP) * A).astype(np.float32)
    Mq = M.astype(np.float16)
    Rs = (np.float32(64.0) * (M - Mq.astype(np.float32))).astype(np.float16)

    def pack(mat):
        """lhsT chunks: chunk (k,m) = mat.T[128k:.., 128m:..]."""
        matT = mat.T
        p = np.empty((128, 4 * 128), dtype=np.float16)
        for k in range(H):
            for m in range(H):
                c = 2 * k + m
                p[:, 128 * c:128 * (c + 1)] = \
                    matT[128 * k:128 * (k + 1), 128 * m:128 * (m + 1)]
        return p

    mq_p = pack(Mq)
    rs_p = pack(Rs)

    bw = BSH // NG
    gw = H * bw
    # byf[p, g, m*bw + b] = by[m*128 + p]
    byf = np.empty((128, NG, gw), dtype=np.float32)
    for m in range(H):
        byf[:, :, m * bw:(m + 1) * bw] = by[m * 128:(m + 1) * 128, 0:1][
            :, None, :]

    def fold(arr):
        """[256, BSH] -> [128, H, BSH]."""
        out = np.empty((128, H, BSH), dtype=arr.dtype)
        for h in range(H):
            out[:, h, :] = arr[128 * h:128 * (h + 1), :]
        return out

    in_maps = []
    for c in range(NCORES):
        Xs = (X0[c * BSH:(c + 1) * BSH, :].T
              / np.float32(STEP)).astype(np.float32)  # [n, bsh]
        X0h = Xs.astype(np.float16)
        X0l = (Xs - X0h.astype(np.float32)).astype(np.float16)
        X0h64 = (X0h.astype(np.float32) / np.float32(64.0)).astype(
            np.float16)
        in_maps.append({
            "mq": mq_p,
            "rs": rs_p,
            "x0h": fold(X0h),
            "x0l": fold(X0l),
            "x0h64": fold(X0h64),
            "byf": byf,
        })
    return in_maps


_CACHED_NC = None


def _get_nc():
    global _CACHED_NC
    if _CACHED_NC is None:
        _CACHED_NC = _build_graph()
    return _CACHED_NC


def kernel(X0, W, by, _trace=False, _return_results=False):
    from concourse.bass_utils import run_bass_kernel_spmd

    nc = _get_nc()
    in_maps = _prep_inputs(X0, W, by)
    res = run_bass_kernel_spmd(nc, in_maps, core_ids=list(range(NCORES)),
                               trace=_trace)

    bw = BSH // NG
    X0 = np.asarray(X0, dtype=np.float32)
    out = np.empty((BS, TMAX, N), dtype=np.float32)
    out[:, 0, :] = X0
    for c in range(NCORES):
        arr = res.results[c]["gout"]  # [128, 999, NG, H*bw] fp16
        # p=partition, t, g=group, (m, b) h-major -> G as (b_total, t, n)
        # column index = g*bw + b ; n = m*128 + p
        arr = arr.reshape(128, NSTEPS, NG, H, bw)
        G = np.transpose(arr, (2, 4, 1, 3, 0)).reshape(BSH, NSTEPS, N)
        G = np.cumsum(G.astype(np.float32), axis=1, dtype=np.float32)
        out[c * BSH:(c + 1) * BSH, 1:, :] = (
            X0[c * BSH:(c + 1) * BSH, None, :] + np.float32(STEP) * G)
    if _return_results:
        return out, res
    return out
